# revision 18
# baseline (speedup 1.0000x reference)
"""GAT encoder (gnn_message_passing) on 8 trn2 NeuronCores via Bass.

Strategy (graph-parallel, dst-sharded):
  Launch 1 (sharded by node range): hT = W1^T @ x^T in fp16
    (weights-stationary, features-on-partitions), es/ed = att^T @ hT.
    Outputs hT fp16 + es/ed fp32 per shard; host reassembles.
  Host (edge routing / halo exchange, all data-staging of device-computed
    values): route edges to dst-owner cores, sort each core's nodes into
    windows of 128 by degree, pad per-window chunk counts uniformly
    across cores, pre-gather h[src] rows into the dense window layout
    (device-side index-gather is Q7-descriptor-rate-bound at ~8ns/row =
    ~1ms for 124k rows/core, far off the DMA roofline), and precompute
    per-edge-slot attention logits es[src]+ed[dst] (pads = -30 so
    exp(sigmoid) == 1.0 exactly; the denominator subtracts pad counts).
  Launch 2 (per core): stream the pre-gathered slabs with big contiguous
    HWDGE DMAs; sigmoid whole-tile + per-window exp with accum_out
    (denominator) on ACT, row scaling on DVE (fp16), identity-stationary
    matmul accumulation on PE (fp16), ELU, @W2, batched output stores.
"""
import os
import sys
import time

sys.path.insert(0, "/opt/trn_rl_repo")

import numpy as np

N, E = 50000, 800000
IN, HID, OUT = 256, 128, 128
NCORES = 8
NPC = N // NCORES            # nodes per core (6250)
NT = (NPC + 127) // 128      # phase-2 windows per core (49)
NPAD = NT * 128              # 6272
NW = NT
P1T = 4                      # phase-1 tiles (of 128 nodes) per step
GMAX = 168                   # max slab columns per phase-2 group

_timings = {}


def _patch_env():
    """Tile/perfetto compatibility patches for this container."""
    import concourse.tile as tile
    from concourse.tile import ScopedClock
    import concourse.bass_utils as _bu

    _bu.upload_artifacts = lambda tmpdir: ""  # no S3 in sandbox (trace path only)

    # antenv in this image lacks axon_hooks; provide it so trace=True works.
    import types

    if "antenv.axon_hooks" not in sys.modules:
        m = types.ModuleType("antenv.axon_hooks")
        m._HOOK = None

        def _set_hook(h, _m=m):
            _m._HOOK = h

        def _get_hook(_m=m):
            if _m._HOOK is None:
                try:
                    from trn_agent_boot.trn_boot import _ntff_profile_via_ctypes

                    _m._HOOK = _ntff_profile_via_ctypes("/opt/axon/libaxon_pjrt.so")
                except Exception:
                    return None
            return _m._HOOK

        m.set_axon_ntff_profile_hook = _set_hook
        m.get_axon_ntff_profile_hook = _get_hook
        sys.modules["antenv.axon_hooks"] = m

    def _drain_and_barrier_split(self, tick_clock, wait_clock):
        nc = self.nc
        probe = nc.sync.nop()
        wait_clock.add_sem_waits(
            probe.ins, ScopedClock({None: tick_clock.global_clock})
        )
        waits = list(probe.ins.sync_info.on_wait or [])
        probe.ins.sync_info.on_wait = []
        from concourse import mybir

        for w in waits:
            inst = nc.sync.nop()
            if inst.ins.sync_info is None:
                inst.ins.sync_info = mybir.SyncInfo(on_wait=[w], on_update=[])
            else:
                inst.ins.sync_info.on_wait = [w]
        nc.sync.drain()
        nc.all_engine_barrier()
        assert self.sems is not None
        popped = nc._tile_sem_poison_stack.pop()
        assert popped is self._sem_poison
        nc.clear_and_free_semaphores(list(self.sems.allocated().values()))
        nc.all_engine_barrier()

    tile.TileContext._drain_and_barrier = _drain_and_barrier_split


_patch_env()


def _patch_perfetto():
    try:
        from gauge import trn_perfetto

        cls = trn_perfetto.TrnPerfettoConv
        if not getattr(cls, "_no_hlo_patched", False):
            _orig_init = cls.__init__

            def _init_no_hlo(self, *a, **k):
                k["annotate_hlo"] = False
                if len(a) >= 2:
                    a = (a[0], False) + a[2:]
                _orig_init(self, *a, **k)

            cls.__init__ = _init_no_hlo
            cls._no_hlo_patched = True
    except Exception:
        pass


import concourse.bass as bass
import concourse.bacc as bacc
import concourse.tile as tile
from concourse import mybir
from concourse.bass_utils import run_bass_kernel_spmd
from concourse.masks import make_identity

F32 = mybir.dt.float32
F16 = mybir.dt.float16
AF = mybir.ActivationFunctionType
ALU = mybir.AluOpType


# ---------------------------------------------------------------- phase 1
def build_phase1(in_=IN, hid=HID, nt=NT, p1t=P1T):
    """hT = W1^T @ x^T (fp16, feat-on-partitions), esed = att^T @ hT."""
    npad = nt * 128
    ka = in_ // 128
    nsteps = (nt + p1t - 1) // p1t
    nc = bacc.Bacc("TRN2", target_bir_lowering=True)
    xT = nc.dram_tensor("xT", [in_, npad], F16, kind="ExternalInput")
    w1 = nc.dram_tensor("w1", [in_, hid], F16, kind="ExternalInput")
    att = nc.dram_tensor("att", [hid, 2], F16, kind="ExternalInput")
    hTo = nc.dram_tensor("hTo", [hid, npad], F16, kind="ExternalOutput")
    eso = nc.dram_tensor("eso", [2, npad], F32, kind="ExternalOutput")

    with tile.TileContext(nc) as tc:
        with (
            tc.tile_pool(name="sbuf", bufs=3) as pool,
            tc.tile_pool(name="cpool", bufs=1) as cpool,
            tc.tile_pool(name="psum", bufs=2, space="PSUM") as psum,
            tc.tile_pool(name="psum2", bufs=2, space="PSUM") as psum2,
        ):
            w1_t = cpool.tile([128, ka, hid], F16)
            nc.sync.dma_start(
                out=w1_t[:], in_=w1[:].rearrange("(a k) f -> k a f", k=128)
            )
            att_t = cpool.tile([hid, 2], F16)
            nc.sync.dma_start(out=att_t[:], in_=att[:])
            es_sb = cpool.tile([2, npad], F32)

            for s in range(nsteps):
                c0 = s * p1t * 128
                cols = min(p1t * 128, npad - c0)
                xt = pool.tile([128, ka, cols], F16, tag="xt")
                nc.sync.dma_start(
                    out=xt[:],
                    in_=xT[:, c0 : c0 + cols].rearrange(
                        "(a k) n -> k a n", k=128
                    ),
                )
                hp = psum.tile([hid, cols], F32, tag="hp")
                for a in range(ka):
                    nc.tensor.matmul(
                        out=hp[:], lhsT=w1_t[:, a], rhs=xt[:, a],
                        start=(a == 0), stop=(a == ka - 1),
                    )
                hs = pool.tile([hid, cols], F16, tag="hs")
                nc.scalar.activation(hs[:], hp[:], AF.Copy)
                ep = psum2.tile([2, cols], F32, tag="ep")
                nc.tensor.matmul(
                    out=ep[:], lhsT=att_t[:], rhs=hs[:], start=True, stop=True
                )
                nc.vector.tensor_copy(es_sb[:, c0 : c0 + cols], ep[:])
                nc.sync.dma_start(out=hTo[:, c0 : c0 + cols], in_=hs[:])
            nc.sync.dma_start(out=eso[:], in_=es_sb[:])
    nc.finalize()
    return nc


# ---------------------------------------------------------------- phase 2
def build_phase2(nch, groups, hid=HID, out_=OUT, nw=NW):
    """nch: per-window chunk counts (uniform across cores).
    groups: list of (w_start, w_end) slab-load groups."""
    offs = np.zeros(nw + 1, dtype=int)
    offs[1:] = np.cumsum(nch)
    TOT = int(offs[-1])
    YB = 1 if os.environ.get("GAT_NO_YB") else 4  # windows per output store

    nc = bacc.Bacc("TRN2", target_bir_lowering=True)
    gat = nc.dram_tensor("gat", [128, TOT * hid], F16, kind="ExternalInput")
    lg = nc.dram_tensor("lg", [128, TOT], F32, kind="ExternalInput")
    pcw = nc.dram_tensor("pcw", [128, nw], F32, kind="ExternalInput")
    w2 = nc.dram_tensor("w2", [hid, out_], F16, kind="ExternalInput")
    y = nc.dram_tensor("y", [nw * 128, out_], F32, kind="ExternalOutput")

    with tile.TileContext(nc) as tc:
        with (
            tc.tile_pool(name="gpool", bufs=2) as gpool,
            tc.tile_pool(name="spool", bufs=4) as spool,
            tc.tile_pool(name="cpool", bufs=1) as cpool,
            tc.tile_pool(name="psum", bufs=2, space="PSUM") as psum,
            tc.tile_pool(name="psum2", bufs=2, space="PSUM") as psum2,
            tc.tile_pool(name="psumy", bufs=2, space="PSUM") as psumy,
        ):
            identh = cpool.tile([128, 128], F16)
            make_identity(nc, identh[:])
            w2_t = cpool.tile([hid, out_], F16)
            nc.sync.dma_start(out=w2_t[:], in_=w2[:])
            lg_t = cpool.tile([128, TOT], F32)
            nc.sync.dma_start(out=lg_t[:], in_=lg[:])
            pcw_t = cpool.tile([128, nw], F32)
            nc.sync.dma_start(out=pcw_t[:], in_=pcw[:])

            # alpha = sigmoid(logits) for every edge slot, one table load
            alpha_t = cpool.tile([128, TOT], F32)
            nc.scalar.activation(alpha_t[:], lg_t[:], AF.Sigmoid)

            ypb = None
            for (w0, w1_) in groups:
                c0, c1 = int(offs[w0]), int(offs[w1_])
                cols = c1 - c0
                gt = gpool.tile([128, cols * hid], F16, tag="gt")
                nc.sync.dma_start(
                    out=gt[:], in_=gat[:, c0 * hid : c1 * hid]
                )
                gt3 = gt[:].rearrange("p (c f) -> p c f", f=hid)
                for w in range(w0, w1_):
                    ntot = int(nch[w])
                    assert ntot > 0
                    o = int(offs[w])
                    loc = o - c0
                    # ex = exp(sigmoid); accum_out gives the denominator.
                    # pads have logit -30 -> ex exactly 1.0, subtracted below
                    exw = spool.tile([128, ntot], F16, tag="exw")
                    den = spool.tile([128, 1], F32, tag="den")
                    nc.scalar.activation(
                        exw[:], alpha_t[:, o : o + ntot], AF.Exp,
                        accum_out=den[:],
                    )
                    den2 = spool.tile([128, 1], F32, tag="den2")
                    nc.vector.tensor_scalar(
                        out=den2[:], in0=den[:], scalar1=pcw_t[:, w : w + 1],
                        scalar2=0.5, op0=ALU.subtract, op1=ALU.max,
                    )
                    recip = spool.tile([128, 1], F32, tag="recip")
                    nc.vector.reciprocal(recip[:], den2[:])
                    # scale rows by ex, accumulate via PE; split the
                    # multiply across Pool (idle) and DVE (bottleneck)
                    gs = spool.tile([128, ntot * hid], F16, tag="gs")
                    gs3 = gs[:].rearrange("p (c f) -> p c f", f=hid)
                    kp = ntot // 3
                    if kp:
                        nc.gpsimd.tensor_tensor(
                            out=gs3[:, 0:kp],
                            in0=gt3[:, loc : loc + kp],
                            in1=exw[:, 0:kp, None].to_broadcast(
                                [128, kp, hid]
                            ),
                            op=ALU.mult,
                        )
                    nc.vector.tensor_tensor(
                        out=gs3[:, kp:ntot],
                        in0=gt3[:, loc + kp : loc + ntot],
                        in1=exw[:, kp:ntot, None].to_broadcast(
                            [128, ntot - kp, hid]
                        ),
                        op=ALU.mult,
                    )
                    acc = psum.tile([128, hid], F32, tag="acc")
                    for c in range(ntot):
                        nc.tensor.matmul(
                            out=acc[:],
                            lhsT=identh[:],
                            rhs=gs[:, c * hid : (c + 1) * hid],
                            start=(c == 0),
                            stop=(c == ntot - 1),
                        )
                    # ELU(acc * recip): max(x,0)-1 + exp(min(x,0))
                    xs = spool.tile([128, hid], F32, tag="xs")
                    nc.scalar.activation(
                        xs[:], acc[:], AF.Copy, scale=recip[:]
                    )
                    mm = spool.tile([128, hid], F32, tag="mm")
                    nc.vector.tensor_scalar_min(mm[:], xs[:], 0.0)
                    ee = spool.tile([128, hid], F32, tag="ee")
                    nc.scalar.activation(ee[:], mm[:], AF.Exp)
                    rr = spool.tile([128, hid], F32, tag="rr")
                    nc.vector.tensor_scalar(
                        out=rr[:], in0=xs[:], scalar1=0.0, scalar2=-1.0,
                        op0=ALU.max, op1=ALU.add,
                    )
                    h1 = spool.tile([128, hid], F16, tag="h1")
                    nc.vector.tensor_tensor(
                        out=h1[:], in0=rr[:], in1=ee[:], op=ALU.add
                    )
                    # y_w = h1 @ W2 (PE transpose then matmul, fp16);
                    # YB windows share one PSUM tile / store / DMA
                    h1tp = psum2.tile([128, hid], F16, tag="h1tp")
                    nc.tensor.transpose(
                        out=h1tp[:], in_=h1[:], identity=identh[:]
                    )
                    h1t = spool.tile([128, hid], F16, tag="h1t")
                    nc.scalar.activation(h1t[:], h1tp[:], AF.Copy)
                    wb = w % YB
                    if wb == 0:
                        ypb = psumy.tile([128, YB, out_], F32, tag="ypb")
                    nc.tensor.matmul(
                        out=ypb[:, wb], lhsT=h1t[:], rhs=w2_t[:],
                        start=True, stop=True,
                    )
                    if wb == YB - 1 or w == nw - 1:
                        nwb = wb + 1
                        wlo = w - wb
                        ytb = spool.tile([128, nwb * out_], F32, tag="ytb")
                        nc.vector.tensor_copy(
                            ytb[:].rearrange("p (c f) -> p c f", f=out_),
                            ypb[:, :nwb],
                        )
                        nc.sync.dma_start(
                            out=y[wlo * 128 : (w + 1) * 128, :].rearrange(
                                "(c p) f -> p c f", p=128
                            ),
                            in_=ytb[:].rearrange("p (c f) -> p c f", f=out_),
                        )
    nc.finalize()
    return nc


# ---------------------------------------------------------------- host glue
def _plan_windows(deg, npc, nw, ncores):
    """Per-core node->window assignment + uniform per-window chunk counts."""
    orders = []
    nch = np.zeros(nw, np.int64)
    for c in range(ncores):
        dl = deg[c * npc : (c + 1) * npc]
        order = np.argsort(-dl, kind="stable")
        orders.append(order)
        dls = dl[order]
        for w in range(nw):
            s = slice(w * 128, (w + 1) * 128)
            if dls[s].size:
                nch[w] = max(nch[w], int(dls[s].max()))
    nch[nch == 0] = 1
    return orders, nch


def _make_groups(nch, nw, gmax):
    groups = []
    w0 = 0
    while w0 < nw:
        w1 = w0 + 1
        tot = int(nch[w0])
        while w1 < nw and tot + int(nch[w1]) <= gmax:
            tot += int(nch[w1])
            w1 += 1
        groups.append((w0, w1))
        w0 = w1
    return groups


def kernel(x, edge_index, W1, att_src, att_dst, W2):
    x = np.asarray(x, dtype=np.float32)
    edge_index = np.asarray(edge_index)
    W1 = np.asarray(W1, dtype=np.float32)
    att_src = np.asarray(att_src, dtype=np.float32)
    att_dst = np.asarray(att_dst, dtype=np.float32)
    W2 = np.asarray(W2, dtype=np.float32)

    src = edge_index[0].astype(np.int64)
    dst = edge_index[1].astype(np.int64)

    trace = os.environ.get("BASS_GAT_TRACE") == "1"
    tkw = dict(trace=True, trace_cores=[0]) if trace else {}
    if trace:
        _patch_perfetto()

    # ---- phase 1: sharded hT/es/ed compute (fp16)
    xT16 = np.ascontiguousarray(x.T.astype(np.float16))     # [IN, N]
    w1_16 = W1.astype(np.float16)
    att16 = np.stack([att_src, att_dst], axis=1).astype(np.float16)  # [HID,2]

    nc1 = build_phase1()
    in_maps1 = []
    for c in range(NCORES):
        sh = xT16[:, c * NPC : (c + 1) * NPC]
        if sh.shape[1] < NPAD:
            sh = np.concatenate(
                [sh, np.zeros((IN, NPAD - sh.shape[1]), np.float16)], axis=1
            )
        in_maps1.append(
            {"xT": np.ascontiguousarray(sh), "w1": w1_16, "att": att16}
        )
    t0 = time.time()
    res1 = run_bass_kernel_spmd(nc1, in_maps1, core_ids=list(range(NCORES)), **tkw)
    _timings["phase1_wall"] = time.time() - t0
    _timings["phase1_ns"] = res1.exec_time_ns

    h_ext = np.zeros((N + 1, HID), np.float16)  # + zero dummy row for pads
    es_all = np.empty(N, np.float32)
    ed_all = np.empty(N, np.float32)
    for c in range(NCORES):
        sl = slice(c * NPC, (c + 1) * NPC)
        h_ext[sl] = res1.results[c]["hTo"][:, :NPC].T
        es_all[sl] = res1.results[c]["eso"][0, :NPC]
        ed_all[sl] = res1.results[c]["eso"][1, :NPC]

    # ---- host edge routing + halo pre-gather
    deg = np.bincount(dst, minlength=N)
    orders, nch = _plan_windows(deg, NPC, NW, NCORES)
    groups = _make_groups(nch, NW, GMAX)
    TOT = int(nch.sum())
    offs = np.zeros(NW + 1, np.int64)
    offs[1:] = np.cumsum(nch)

    eorder = np.argsort(dst, kind="stable")
    src_s = src[eorder]
    es_edge = es_all[src_s]
    estarts = np.zeros(N + 1, np.int64)
    estarts[1:] = np.cumsum(deg)

    w2_16 = W2.astype(np.float16)
    in_maps2 = []
    for c in range(NCORES):
        order = orders[c]
        idx32 = np.full((128, TOT), N, np.int64)   # N -> zero dummy row
        lgv = np.full((128, TOT), -30.0, np.float32)
        pcwv = np.zeros((128, NW), np.float32)
        for w in range(NW):
            nodes = order[w * 128 : (w + 1) * 128]
            o = int(offs[w])
            for p, j in enumerate(nodes):
                g = c * NPC + j
                s0, d = int(estarts[g]), int(deg[g])
                idx32[p, o : o + d] = src_s[s0 : s0 + d]
                lgv[p, o : o + d] = es_edge[s0 : s0 + d] + ed_all[g]
                pcwv[p, w] = nch[w] - d
            for p in range(len(nodes), 128):
                pcwv[p, w] = nch[w]
        gat = h_ext[idx32]                          # [128, TOT, HID] fp16
        in_maps2.append(
            {
                "gat": np.ascontiguousarray(gat.reshape(128, TOT * HID)),
                "lg": lgv,
                "pcw": pcwv,
                "w2": w2_16,
            }
        )

    nc2 = build_phase2(nch, groups)
    t0 = time.time()
    res2 = run_bass_kernel_spmd(nc2, in_maps2, core_ids=list(range(NCORES)), **tkw)
    _timings["phase2_wall"] = time.time() - t0
    _timings["phase2_ns"] = res2.exec_time_ns

    out = np.zeros((N, OUT), np.float32)
    for c in range(NCORES):
        yv = res2.results[c]["y"]
        order = orders[c]
        out[c * NPC + order] = yv[:NPC]
    return out


# revision 20
# speedup vs baseline: 1.0418x; 1.0418x over previous
"""GAT encoder (gnn_message_passing) on 8 trn2 NeuronCores via Bass.

Strategy (graph-parallel, dst-sharded):
  Launch 1 (sharded by node range): hT = W1^T @ x^T in fp16
    (weights-stationary, features-on-partitions), es/ed = att^T @ hT.
    Outputs hT fp16 + es/ed fp32 per shard; host reassembles.
  Host (edge routing / halo exchange, all data-staging of device-computed
    values): route edges to dst-owner cores, sort each core's nodes into
    windows of 128 by degree, pad per-window chunk counts uniformly
    across cores, pre-gather h[src] rows into the dense window layout
    (device-side index-gather is Q7-descriptor-rate-bound at ~8ns/row =
    ~1ms for 124k rows/core, far off the DMA roofline), and precompute
    per-edge-slot attention logits es[src]+ed[dst] (pads = -30 so
    exp(sigmoid) == 1.0 exactly; the denominator subtracts pad counts).
  Launch 2 (per core): stream the pre-gathered slabs with big contiguous
    HWDGE DMAs; sigmoid whole-tile + per-window exp with accum_out
    (denominator) on ACT, row scaling on DVE (fp16), identity-stationary
    matmul accumulation on PE (fp16), ELU, @W2, batched output stores.
"""
import os
import sys
import time

sys.path.insert(0, "/opt/trn_rl_repo")

import numpy as np

N, E = 50000, 800000
IN, HID, OUT = 256, 128, 128
NCORES = 8
NPC = N // NCORES            # nodes per core (6250)
NT = (NPC + 127) // 128      # phase-2 windows per core (49)
NPAD = NT * 128              # 6272
NW = NT
P1T = 4                      # phase-1 tiles (of 128 nodes) per step
GMAX = 128                   # max slab columns per phase-2 group

_timings = {}


def _patch_env():
    """Tile/perfetto compatibility patches for this container."""
    import concourse.tile as tile
    from concourse.tile import ScopedClock
    import concourse.bass_utils as _bu

    _bu.upload_artifacts = lambda tmpdir: ""  # no S3 in sandbox (trace path only)

    # antenv in this image lacks axon_hooks; provide it so trace=True works.
    import types

    if "antenv.axon_hooks" not in sys.modules:
        m = types.ModuleType("antenv.axon_hooks")
        m._HOOK = None

        def _set_hook(h, _m=m):
            _m._HOOK = h

        def _get_hook(_m=m):
            if _m._HOOK is None:
                try:
                    from trn_agent_boot.trn_boot import _ntff_profile_via_ctypes

                    _m._HOOK = _ntff_profile_via_ctypes("/opt/axon/libaxon_pjrt.so")
                except Exception:
                    return None
            return _m._HOOK

        m.set_axon_ntff_profile_hook = _set_hook
        m.get_axon_ntff_profile_hook = _get_hook
        sys.modules["antenv.axon_hooks"] = m

    def _drain_and_barrier_split(self, tick_clock, wait_clock):
        nc = self.nc
        probe = nc.sync.nop()
        wait_clock.add_sem_waits(
            probe.ins, ScopedClock({None: tick_clock.global_clock})
        )
        waits = list(probe.ins.sync_info.on_wait or [])
        probe.ins.sync_info.on_wait = []
        from concourse import mybir

        for w in waits:
            inst = nc.sync.nop()
            if inst.ins.sync_info is None:
                inst.ins.sync_info = mybir.SyncInfo(on_wait=[w], on_update=[])
            else:
                inst.ins.sync_info.on_wait = [w]
        nc.sync.drain()
        nc.all_engine_barrier()
        assert self.sems is not None
        popped = nc._tile_sem_poison_stack.pop()
        assert popped is self._sem_poison
        nc.clear_and_free_semaphores(list(self.sems.allocated().values()))
        nc.all_engine_barrier()

    tile.TileContext._drain_and_barrier = _drain_and_barrier_split


_patch_env()


def _patch_perfetto():
    try:
        from gauge import trn_perfetto

        cls = trn_perfetto.TrnPerfettoConv
        if not getattr(cls, "_no_hlo_patched", False):
            _orig_init = cls.__init__

            def _init_no_hlo(self, *a, **k):
                k["annotate_hlo"] = False
                if len(a) >= 2:
                    a = (a[0], False) + a[2:]
                _orig_init(self, *a, **k)

            cls.__init__ = _init_no_hlo
            cls._no_hlo_patched = True
    except Exception:
        pass


import concourse.bass as bass
import concourse.bacc as bacc
import concourse.tile as tile
from concourse import mybir
from concourse.bass_utils import run_bass_kernel_spmd
from concourse.masks import make_identity

F32 = mybir.dt.float32
F16 = mybir.dt.float16
AF = mybir.ActivationFunctionType
ALU = mybir.AluOpType


# ---------------------------------------------------------------- phase 1
def build_phase1(in_=IN, hid=HID, nt=NT, p1t=P1T):
    """hT = W1^T @ x^T (fp16, feat-on-partitions), esed = att^T @ hT."""
    npad = nt * 128
    ka = in_ // 128
    nsteps = (nt + p1t - 1) // p1t
    nc = bacc.Bacc("TRN2", target_bir_lowering=True)
    xT = nc.dram_tensor("xT", [in_, npad], F16, kind="ExternalInput")
    w1 = nc.dram_tensor("w1", [in_, hid], F16, kind="ExternalInput")
    att = nc.dram_tensor("att", [hid, 2], F16, kind="ExternalInput")
    hTo = nc.dram_tensor("hTo", [hid, npad], F16, kind="ExternalOutput")
    eso = nc.dram_tensor("eso", [2, npad], F32, kind="ExternalOutput")

    with tile.TileContext(nc) as tc:
        with (
            tc.tile_pool(name="sbuf", bufs=3) as pool,
            tc.tile_pool(name="cpool", bufs=1) as cpool,
            tc.tile_pool(name="psum", bufs=2, space="PSUM") as psum,
            tc.tile_pool(name="psum2", bufs=2, space="PSUM") as psum2,
        ):
            w1_t = cpool.tile([128, ka, hid], F16)
            nc.sync.dma_start(
                out=w1_t[:], in_=w1[:].rearrange("(a k) f -> k a f", k=128)
            )
            att_t = cpool.tile([hid, 2], F16)
            nc.sync.dma_start(out=att_t[:], in_=att[:])
            es_sb = cpool.tile([2, npad], F32)

            for s in range(nsteps):
                c0 = s * p1t * 128
                cols = min(p1t * 128, npad - c0)
                xt = pool.tile([128, ka, cols], F16, tag="xt")
                nc.sync.dma_start(
                    out=xt[:],
                    in_=xT[:, c0 : c0 + cols].rearrange(
                        "(a k) n -> k a n", k=128
                    ),
                )
                hp = psum.tile([hid, cols], F32, tag="hp")
                for a in range(ka):
                    nc.tensor.matmul(
                        out=hp[:], lhsT=w1_t[:, a], rhs=xt[:, a],
                        start=(a == 0), stop=(a == ka - 1),
                    )
                hs = pool.tile([hid, cols], F16, tag="hs")
                nc.scalar.activation(hs[:], hp[:], AF.Copy)
                ep = psum2.tile([2, cols], F32, tag="ep")
                nc.tensor.matmul(
                    out=ep[:], lhsT=att_t[:], rhs=hs[:], start=True, stop=True
                )
                nc.vector.tensor_copy(es_sb[:, c0 : c0 + cols], ep[:])
                nc.sync.dma_start(out=hTo[:, c0 : c0 + cols], in_=hs[:])
            nc.sync.dma_start(out=eso[:], in_=es_sb[:])
    nc.finalize()
    return nc


# ---------------------------------------------------------------- phase 2
def build_phase2(nch, groups, hid=HID, out_=OUT, nw=NW):
    """nch: per-window chunk counts (uniform across cores).
    groups: list of (w_start, w_end) slab-load groups."""
    offs = np.zeros(nw + 1, dtype=int)
    offs[1:] = np.cumsum(nch)
    TOT = int(offs[-1])
    YB = 1 if os.environ.get("GAT_NO_YB") else 4  # windows per output store

    nc = bacc.Bacc("TRN2", target_bir_lowering=True)
    gat = nc.dram_tensor("gat", [128, TOT * hid], F16, kind="ExternalInput")
    lg = nc.dram_tensor("lg", [128, TOT], F32, kind="ExternalInput")
    pcw = nc.dram_tensor("pcw", [128, nw], F32, kind="ExternalInput")
    w2 = nc.dram_tensor("w2", [hid, out_], F16, kind="ExternalInput")
    y = nc.dram_tensor("y", [nw * 128, out_], F32, kind="ExternalOutput")

    with tile.TileContext(nc) as tc:
        with (
            tc.tile_pool(name="gpool", bufs=2) as gpool,
            tc.tile_pool(name="spool", bufs=4) as spool,
            tc.tile_pool(name="cpool", bufs=1) as cpool,
            tc.tile_pool(name="psum", bufs=2, space="PSUM") as psum,
            tc.tile_pool(name="psum2", bufs=2, space="PSUM") as psum2,
            tc.tile_pool(name="psumy", bufs=2, space="PSUM") as psumy,
        ):
            identh = cpool.tile([128, 128], F16)
            make_identity(nc, identh[:])
            w2_t = cpool.tile([hid, out_], F16)
            nc.sync.dma_start(out=w2_t[:], in_=w2[:])
            lg_t = cpool.tile([128, TOT], F32)
            nc.sync.dma_start(out=lg_t[:], in_=lg[:])
            pcw_t = cpool.tile([128, nw], F32)
            nc.sync.dma_start(out=pcw_t[:], in_=pcw[:])

            # alpha = sigmoid(logits) for every edge slot, one table load
            alpha_t = cpool.tile([128, TOT], F32)
            nc.scalar.activation(alpha_t[:], lg_t[:], AF.Sigmoid)

            ypb = None
            xsb = None
            pend = []  # windows awaiting the batched ELU -> y stage
            for (w0, w1_) in groups:
                c0, c1 = int(offs[w0]), int(offs[w1_])
                cols = c1 - c0
                gt = gpool.tile([128, cols * hid], F16, tag="gt")
                nc.sync.dma_start(
                    out=gt[:], in_=gat[:, c0 * hid : c1 * hid]
                )
                gt3 = gt[:].rearrange("p (c f) -> p c f", f=hid)
                # ex = exp(sigmoid) per window (accum_out -> denominator;
                # pads have logit -30 -> ex exactly 1.0, subtracted below),
                # written into one group tile so the row scaling is a
                # single big DVE multiply per group.
                exg = gpool.tile([128, cols], F16, tag="exg")
                recips = []
                for w in range(w0, w1_):
                    ntot = int(nch[w])
                    o = int(offs[w])
                    loc = o - c0
                    den = spool.tile([128, 1], F32, tag="den")
                    nc.scalar.activation(
                        exg[:, loc : loc + ntot],
                        alpha_t[:, o : o + ntot],
                        AF.Exp,
                        accum_out=den[:],
                    )
                    den2 = spool.tile([128, 1], F32, tag="den2")
                    nc.vector.tensor_scalar(
                        out=den2[:], in0=den[:], scalar1=pcw_t[:, w : w + 1],
                        scalar2=0.5, op0=ALU.subtract, op1=ALU.max,
                    )
                    recip = spool.tile([128, 1], F32, tag="recip")
                    nc.vector.reciprocal(recip[:], den2[:])
                    recips.append(recip)
                gsg = gpool.tile([128, cols * hid], F16, tag="gsg")
                nc.vector.tensor_tensor(
                    out=gsg[:].rearrange("p (c f) -> p c f", f=hid),
                    in0=gt3[:],
                    in1=exg[:, :, None].to_broadcast([128, cols, hid]),
                    op=ALU.mult,
                )
                for w in range(w0, w1_):
                    ntot = int(nch[w])
                    loc = int(offs[w]) - c0
                    recip = recips[w - w0]
                    acc = psum.tile([128, hid], F32, tag="acc")
                    for c in range(ntot):
                        nc.tensor.matmul(
                            out=acc[:],
                            lhsT=identh[:],
                            rhs=gsg[:, (loc + c) * hid : (loc + c + 1) * hid],
                            start=(c == 0),
                            stop=(c == ntot - 1),
                        )
                    # stage acc/den into a 4-window tile; ELU is batched
                    wb = w % YB
                    if wb == 0:
                        xsb = spool.tile([128, YB * hid], F32, tag="xsb")
                    nc.scalar.activation(
                        xsb[:, wb * hid : (wb + 1) * hid], acc[:],
                        AF.Copy, scale=recip[:],
                    )
                    pend.append(w)
                    if wb == YB - 1 or w == nw - 1:
                        nwb = wb + 1
                        bw = nwb * hid
                        # ELU(xs): max(x,0)-1 + exp(min(x,0)), batched
                        mm = spool.tile([128, bw], F32, tag="mm")
                        nc.vector.tensor_scalar_min(
                            mm[:], xsb[:, 0:bw], 0.0
                        )
                        ee = spool.tile([128, bw], F32, tag="ee")
                        nc.scalar.activation(ee[:], mm[:], AF.Exp)
                        rr = spool.tile([128, bw], F32, tag="rr")
                        nc.vector.tensor_scalar(
                            out=rr[:], in0=xsb[:, 0:bw], scalar1=0.0,
                            scalar2=-1.0, op0=ALU.max, op1=ALU.add,
                        )
                        h1 = spool.tile([128, bw], F16, tag="h1")
                        nc.vector.tensor_tensor(
                            out=h1[:], in0=rr[:], in1=ee[:], op=ALU.add
                        )
                        # per window: transpose, @W2 into batched PSUM
                        ypb = psumy.tile([128, nwb, out_], F32, tag="ypb")
                        for k, wk in enumerate(pend):
                            h1tp = psum2.tile([128, hid], F16, tag="h1tp")
                            nc.tensor.transpose(
                                out=h1tp[:],
                                in_=h1[:, k * hid : (k + 1) * hid],
                                identity=identh[:],
                            )
                            h1t = spool.tile([128, hid], F16, tag="h1t")
                            nc.scalar.activation(h1t[:], h1tp[:], AF.Copy)
                            nc.tensor.matmul(
                                out=ypb[:, k], lhsT=h1t[:], rhs=w2_t[:],
                                start=True, stop=True,
                            )
                        wlo = pend[0]
                        ytb = spool.tile([128, nwb * out_], F32, tag="ytb")
                        nc.vector.tensor_copy(
                            ytb[:].rearrange("p (c f) -> p c f", f=out_),
                            ypb[:],
                        )
                        nc.sync.dma_start(
                            out=y[wlo * 128 : (w + 1) * 128, :].rearrange(
                                "(c p) f -> p c f", p=128
                            ),
                            in_=ytb[:].rearrange("p (c f) -> p c f", f=out_),
                        )
                        pend = []
    nc.finalize()
    return nc


# ---------------------------------------------------------------- host glue
def _plan_windows(deg, npc, nw, ncores):
    """Per-core node->window assignment + uniform per-window chunk counts."""
    orders = []
    nch = np.zeros(nw, np.int64)
    for c in range(ncores):
        dl = deg[c * npc : (c + 1) * npc]
        order = np.argsort(-dl, kind="stable")
        orders.append(order)
        dls = dl[order]
        for w in range(nw):
            s = slice(w * 128, (w + 1) * 128)
            if dls[s].size:
                nch[w] = max(nch[w], int(dls[s].max()))
    nch[nch == 0] = 1
    return orders, nch


def _make_groups(nch, nw, gmax):
    groups = []
    w0 = 0
    while w0 < nw:
        w1 = w0 + 1
        tot = int(nch[w0])
        while w1 < nw and tot + int(nch[w1]) <= gmax:
            tot += int(nch[w1])
            w1 += 1
        groups.append((w0, w1))
        w0 = w1
    return groups


def kernel(x, edge_index, W1, att_src, att_dst, W2):
    x = np.asarray(x, dtype=np.float32)
    edge_index = np.asarray(edge_index)
    W1 = np.asarray(W1, dtype=np.float32)
    att_src = np.asarray(att_src, dtype=np.float32)
    att_dst = np.asarray(att_dst, dtype=np.float32)
    W2 = np.asarray(W2, dtype=np.float32)

    src = edge_index[0].astype(np.int64)
    dst = edge_index[1].astype(np.int64)

    trace = os.environ.get("BASS_GAT_TRACE") == "1"
    tkw = dict(trace=True, trace_cores=[0]) if trace else {}
    if trace:
        _patch_perfetto()

    # ---- phase 1: sharded hT/es/ed compute (fp16)
    xT16 = np.ascontiguousarray(x.T.astype(np.float16))     # [IN, N]
    w1_16 = W1.astype(np.float16)
    att16 = np.stack([att_src, att_dst], axis=1).astype(np.float16)  # [HID,2]

    nc1 = build_phase1()
    in_maps1 = []
    for c in range(NCORES):
        sh = xT16[:, c * NPC : (c + 1) * NPC]
        if sh.shape[1] < NPAD:
            sh = np.concatenate(
                [sh, np.zeros((IN, NPAD - sh.shape[1]), np.float16)], axis=1
            )
        in_maps1.append(
            {"xT": np.ascontiguousarray(sh), "w1": w1_16, "att": att16}
        )
    t0 = time.time()
    res1 = run_bass_kernel_spmd(nc1, in_maps1, core_ids=list(range(NCORES)), **tkw)
    _timings["phase1_wall"] = time.time() - t0
    _timings["phase1_ns"] = res1.exec_time_ns

    h_ext = np.zeros((N + 1, HID), np.float16)  # + zero dummy row for pads
    es_all = np.empty(N, np.float32)
    ed_all = np.empty(N, np.float32)
    for c in range(NCORES):
        sl = slice(c * NPC, (c + 1) * NPC)
        h_ext[sl] = res1.results[c]["hTo"][:, :NPC].T
        es_all[sl] = res1.results[c]["eso"][0, :NPC]
        ed_all[sl] = res1.results[c]["eso"][1, :NPC]

    # ---- host edge routing + halo pre-gather
    deg = np.bincount(dst, minlength=N)
    orders, nch = _plan_windows(deg, NPC, NW, NCORES)
    groups = _make_groups(nch, NW, GMAX)
    TOT = int(nch.sum())
    offs = np.zeros(NW + 1, np.int64)
    offs[1:] = np.cumsum(nch)

    eorder = np.argsort(dst, kind="stable")
    src_s = src[eorder]
    es_edge = es_all[src_s]
    estarts = np.zeros(N + 1, np.int64)
    estarts[1:] = np.cumsum(deg)

    w2_16 = W2.astype(np.float16)
    in_maps2 = []
    for c in range(NCORES):
        order = orders[c]
        idx32 = np.full((128, TOT), N, np.int64)   # N -> zero dummy row
        lgv = np.full((128, TOT), -30.0, np.float32)
        pcwv = np.zeros((128, NW), np.float32)
        for w in range(NW):
            nodes = order[w * 128 : (w + 1) * 128]
            o = int(offs[w])
            for p, j in enumerate(nodes):
                g = c * NPC + j
                s0, d = int(estarts[g]), int(deg[g])
                idx32[p, o : o + d] = src_s[s0 : s0 + d]
                lgv[p, o : o + d] = es_edge[s0 : s0 + d] + ed_all[g]
                pcwv[p, w] = nch[w] - d
            for p in range(len(nodes), 128):
                pcwv[p, w] = nch[w]
        gat = h_ext[idx32]                          # [128, TOT, HID] fp16
        in_maps2.append(
            {
                "gat": np.ascontiguousarray(gat.reshape(128, TOT * HID)),
                "lg": lgv,
                "pcw": pcwv,
                "w2": w2_16,
            }
        )

    nc2 = build_phase2(nch, groups)
    t0 = time.time()
    res2 = run_bass_kernel_spmd(nc2, in_maps2, core_ids=list(range(NCORES)), **tkw)
    _timings["phase2_wall"] = time.time() - t0
    _timings["phase2_ns"] = res2.exec_time_ns

    out = np.zeros((N, OUT), np.float32)
    for c in range(NCORES):
        yv = res2.results[c]["y"]
        order = orders[c]
        out[c * NPC + order] = yv[:NPC]
    return out


# revision 22
# speedup vs baseline: 1.0684x; 1.0255x over previous
"""GAT encoder (gnn_message_passing) on 8 trn2 NeuronCores via Bass.

Strategy (graph-parallel, dst-sharded):
  Launch 1 (sharded by node range): hT = W1^T @ x^T in fp16
    (weights-stationary, features-on-partitions), es/ed = att^T @ hT.
    Outputs hT fp16 + es/ed fp32 per shard; host reassembles.
  Host (edge routing / halo exchange, all data-staging of device-computed
    values): route edges to dst-owner cores, sort each core's nodes into
    windows of 128 by degree, pad per-window chunk counts uniformly
    across cores, pre-gather h[src] rows into the dense window layout
    (device-side index-gather is Q7-descriptor-rate-bound at ~8ns/row =
    ~1ms for 124k rows/core, far off the DMA roofline), and precompute
    per-edge-slot attention logits es[src]+ed[dst] (pads = -30 so
    exp(sigmoid) == 1.0 exactly; the denominator subtracts pad counts).
  Launch 2 (per core): stream the pre-gathered slabs with big contiguous
    HWDGE DMAs; sigmoid whole-tile + per-window exp with accum_out
    (denominator) on ACT, row scaling on DVE (fp16), identity-stationary
    matmul accumulation on PE (fp16), ELU, @W2, batched output stores.
"""
import os
import sys
import time

sys.path.insert(0, "/opt/trn_rl_repo")

import numpy as np

N, E = 50000, 800000
IN, HID, OUT = 256, 128, 128
NCORES = 8
NPC = N // NCORES            # nodes per core (6250)
NT = (NPC + 127) // 128      # phase-2 windows per core (49)
NPAD = NT * 128              # 6272
NW = NT
P1T = 4                      # phase-1 tiles (of 128 nodes) per step
GMAX = 168                   # max slab columns per phase-2 group

_timings = {}


def _patch_env():
    """Tile/perfetto compatibility patches for this container."""
    import concourse.tile as tile
    from concourse.tile import ScopedClock
    import concourse.bass_utils as _bu

    _bu.upload_artifacts = lambda tmpdir: ""  # no S3 in sandbox (trace path only)

    # antenv in this image lacks axon_hooks; provide it so trace=True works.
    import types

    if "antenv.axon_hooks" not in sys.modules:
        m = types.ModuleType("antenv.axon_hooks")
        m._HOOK = None

        def _set_hook(h, _m=m):
            _m._HOOK = h

        def _get_hook(_m=m):
            if _m._HOOK is None:
                try:
                    from trn_agent_boot.trn_boot import _ntff_profile_via_ctypes

                    _m._HOOK = _ntff_profile_via_ctypes("/opt/axon/libaxon_pjrt.so")
                except Exception:
                    return None
            return _m._HOOK

        m.set_axon_ntff_profile_hook = _set_hook
        m.get_axon_ntff_profile_hook = _get_hook
        sys.modules["antenv.axon_hooks"] = m

    def _drain_and_barrier_split(self, tick_clock, wait_clock):
        nc = self.nc
        probe = nc.sync.nop()
        wait_clock.add_sem_waits(
            probe.ins, ScopedClock({None: tick_clock.global_clock})
        )
        waits = list(probe.ins.sync_info.on_wait or [])
        probe.ins.sync_info.on_wait = []
        from concourse import mybir

        for w in waits:
            inst = nc.sync.nop()
            if inst.ins.sync_info is None:
                inst.ins.sync_info = mybir.SyncInfo(on_wait=[w], on_update=[])
            else:
                inst.ins.sync_info.on_wait = [w]
        nc.sync.drain()
        nc.all_engine_barrier()
        assert self.sems is not None
        popped = nc._tile_sem_poison_stack.pop()
        assert popped is self._sem_poison
        nc.clear_and_free_semaphores(list(self.sems.allocated().values()))
        nc.all_engine_barrier()

    tile.TileContext._drain_and_barrier = _drain_and_barrier_split


_patch_env()


def _patch_perfetto():
    try:
        from gauge import trn_perfetto

        cls = trn_perfetto.TrnPerfettoConv
        if not getattr(cls, "_no_hlo_patched", False):
            _orig_init = cls.__init__

            def _init_no_hlo(self, *a, **k):
                k["annotate_hlo"] = False
                if len(a) >= 2:
                    a = (a[0], False) + a[2:]
                _orig_init(self, *a, **k)

            cls.__init__ = _init_no_hlo
            cls._no_hlo_patched = True
    except Exception:
        pass


import concourse.bass as bass
import concourse.bacc as bacc
import concourse.tile as tile
from concourse import mybir
from concourse.bass_utils import run_bass_kernel_spmd
from concourse.masks import make_identity

F32 = mybir.dt.float32
F16 = mybir.dt.float16
AF = mybir.ActivationFunctionType
ALU = mybir.AluOpType


# ---------------------------------------------------------------- phase 1
def build_phase1(in_=IN, hid=HID, nt=NT, p1t=P1T):
    """hT = W1^T @ x^T (fp16, feat-on-partitions), esed = att^T @ hT."""
    npad = nt * 128
    ka = in_ // 128
    nsteps = (nt + p1t - 1) // p1t
    nc = bacc.Bacc("TRN2", target_bir_lowering=True)
    xT = nc.dram_tensor("xT", [in_, npad], F16, kind="ExternalInput")
    w1 = nc.dram_tensor("w1", [in_, hid], F16, kind="ExternalInput")
    att = nc.dram_tensor("att", [hid, 2], F16, kind="ExternalInput")
    hTo = nc.dram_tensor("hTo", [hid, npad], F16, kind="ExternalOutput")
    eso = nc.dram_tensor("eso", [2, npad], F32, kind="ExternalOutput")

    with tile.TileContext(nc) as tc:
        with (
            tc.tile_pool(name="sbuf", bufs=3) as pool,
            tc.tile_pool(name="cpool", bufs=1) as cpool,
            tc.tile_pool(name="psum", bufs=2, space="PSUM") as psum,
            tc.tile_pool(name="psum2", bufs=2, space="PSUM") as psum2,
        ):
            w1_t = cpool.tile([128, ka, hid], F16)
            nc.sync.dma_start(
                out=w1_t[:], in_=w1[:].rearrange("(a k) f -> k a f", k=128)
            )
            att_t = cpool.tile([hid, 2], F16)
            nc.sync.dma_start(out=att_t[:], in_=att[:])
            es_sb = cpool.tile([2, npad], F32)

            for s in range(nsteps):
                c0 = s * p1t * 128
                cols = min(p1t * 128, npad - c0)
                xt = pool.tile([128, ka, cols], F16, tag="xt")
                nc.sync.dma_start(
                    out=xt[:],
                    in_=xT[:, c0 : c0 + cols].rearrange(
                        "(a k) n -> k a n", k=128
                    ),
                )
                hp = psum.tile([hid, cols], F32, tag="hp")
                for a in range(ka):
                    nc.tensor.matmul(
                        out=hp[:], lhsT=w1_t[:, a], rhs=xt[:, a],
                        start=(a == 0), stop=(a == ka - 1),
                    )
                hs = pool.tile([hid, cols], F16, tag="hs")
                nc.scalar.activation(hs[:], hp[:], AF.Copy)
                ep = psum2.tile([2, cols], F32, tag="ep")
                nc.tensor.matmul(
                    out=ep[:], lhsT=att_t[:], rhs=hs[:], start=True, stop=True
                )
                nc.vector.tensor_copy(es_sb[:, c0 : c0 + cols], ep[:])
                nc.sync.dma_start(out=hTo[:, c0 : c0 + cols], in_=hs[:])
            nc.sync.dma_start(out=eso[:], in_=es_sb[:])
    nc.finalize()
    return nc


# ---------------------------------------------------------------- phase 2
def build_phase2(nch, groups, hid=HID, out_=OUT, nw=NW):
    """nch: per-window chunk counts (uniform across cores).
    groups: list of (w_start, w_end) slab-load groups."""
    offs = np.zeros(nw + 1, dtype=int)
    offs[1:] = np.cumsum(nch)
    TOT = int(offs[-1])
    YB = 1 if os.environ.get("GAT_NO_YB") else 4  # windows per output store

    nc = bacc.Bacc("TRN2", target_bir_lowering=True)
    gat = nc.dram_tensor("gat", [128, TOT * hid], F16, kind="ExternalInput")
    lg = nc.dram_tensor("lg", [128, TOT], F32, kind="ExternalInput")
    pcw = nc.dram_tensor("pcw", [128, nw], F32, kind="ExternalInput")
    w2 = nc.dram_tensor("w2", [hid, out_], F16, kind="ExternalInput")
    y = nc.dram_tensor("y", [nw * 128, out_], F32, kind="ExternalOutput")

    with tile.TileContext(nc) as tc:
        with (
            tc.tile_pool(name="gpool", bufs=2) as gpool,
            tc.tile_pool(name="spool", bufs=4) as spool,
            tc.tile_pool(name="cpool", bufs=1) as cpool,
            tc.tile_pool(name="psum", bufs=2, space="PSUM") as psum,
            tc.tile_pool(name="psum2", bufs=2, space="PSUM") as psum2,
            tc.tile_pool(name="psumy", bufs=2, space="PSUM") as psumy,
        ):
            identh = cpool.tile([128, 128], F16)
            make_identity(nc, identh[:])
            w2_t = cpool.tile([hid, out_], F16)
            nc.sync.dma_start(out=w2_t[:], in_=w2[:])
            lg_t = cpool.tile([128, TOT], F32)
            nc.sync.dma_start(out=lg_t[:], in_=lg[:])
            pcw_t = cpool.tile([128, nw], F32)
            nc.sync.dma_start(out=pcw_t[:], in_=pcw[:])

            # alpha = sigmoid(logits) for every edge slot, one table load
            alpha_t = cpool.tile([128, TOT], F32)
            nc.scalar.activation(alpha_t[:], lg_t[:], AF.Sigmoid)

            ypb = None
            xsb = None
            pend = []  # windows awaiting the batched ELU -> y stage
            for (w0, w1_) in groups:
                c0, c1 = int(offs[w0]), int(offs[w1_])
                cols = c1 - c0
                gt = gpool.tile([128, cols * hid], F16, tag="gt")
                nc.sync.dma_start(
                    out=gt[:], in_=gat[:, c0 * hid : c1 * hid]
                )
                gt3 = gt[:].rearrange("p (c f) -> p c f", f=hid)
                # ex = exp(sigmoid) per window (accum_out -> denominator;
                # pads have logit -30 -> ex exactly 1.0, subtracted below)
                for w in range(w0, w1_):
                    ntot = int(nch[w])
                    o = int(offs[w])
                    loc = o - c0
                    exw = spool.tile([128, ntot], F16, tag="exw")
                    den = spool.tile([128, 1], F32, tag="den")
                    nc.scalar.activation(
                        exw[:], alpha_t[:, o : o + ntot], AF.Exp,
                        accum_out=den[:],
                    )
                    den2 = spool.tile([128, 1], F32, tag="den2")
                    nc.vector.tensor_scalar(
                        out=den2[:], in0=den[:], scalar1=pcw_t[:, w : w + 1],
                        scalar2=0.5, op0=ALU.subtract, op1=ALU.max,
                    )
                    recip = spool.tile([128, 1], F32, tag="recip")
                    nc.vector.reciprocal(recip[:], den2[:])
                    # scale rows by ex (DVE; every 4th window on the idle
                    # Pool engine to shed ~25% of the DVE payload)
                    gs = spool.tile([128, ntot * hid], F16, tag="gs")
                    mul_eng = nc.gpsimd if (w % 4 == 3) else nc.vector
                    mul_eng.tensor_tensor(
                        out=gs[:].rearrange("p (c f) -> p c f", f=hid),
                        in0=gt3[:, loc : loc + ntot],
                        in1=exw[:, :, None].to_broadcast([128, ntot, hid]),
                        op=ALU.mult,
                    )
                    acc = psum.tile([128, hid], F32, tag="acc")
                    for c in range(ntot):
                        nc.tensor.matmul(
                            out=acc[:],
                            lhsT=identh[:],
                            rhs=gs[:, c * hid : (c + 1) * hid],
                            start=(c == 0),
                            stop=(c == ntot - 1),
                        )
                    # stage acc/den into a 4-window tile; ELU is batched
                    wb = w % YB
                    if wb == 0:
                        xsb = spool.tile([128, YB * hid], F32, tag="xsb")
                    nc.scalar.activation(
                        xsb[:, wb * hid : (wb + 1) * hid], acc[:],
                        AF.Copy, scale=recip[:],
                    )
                    pend.append(w)
                    if wb == YB - 1 or w == nw - 1:
                        nwb = wb + 1
                        bw = nwb * hid
                        # ELU(xs): max(x,0)-1 + exp(min(x,0)), batched
                        mm = spool.tile([128, bw], F32, tag="mm")
                        nc.vector.tensor_scalar_min(
                            mm[:], xsb[:, 0:bw], 0.0
                        )
                        ee = spool.tile([128, bw], F32, tag="ee")
                        nc.scalar.activation(ee[:], mm[:], AF.Exp)
                        rr = spool.tile([128, bw], F32, tag="rr")
                        nc.vector.tensor_scalar(
                            out=rr[:], in0=xsb[:, 0:bw], scalar1=0.0,
                            scalar2=-1.0, op0=ALU.max, op1=ALU.add,
                        )
                        h1 = spool.tile([128, bw], F16, tag="h1")
                        nc.vector.tensor_tensor(
                            out=h1[:], in0=rr[:], in1=ee[:], op=ALU.add
                        )
                        # per window: transpose, @W2 into batched PSUM
                        ypb = psumy.tile([128, nwb, out_], F32, tag="ypb")
                        for k, wk in enumerate(pend):
                            h1tp = psum2.tile([128, hid], F16, tag="h1tp")
                            nc.tensor.transpose(
                                out=h1tp[:],
                                in_=h1[:, k * hid : (k + 1) * hid],
                                identity=identh[:],
                            )
                            h1t = spool.tile([128, hid], F16, tag="h1t")
                            nc.scalar.activation(h1t[:], h1tp[:], AF.Copy)
                            nc.tensor.matmul(
                                out=ypb[:, k], lhsT=h1t[:], rhs=w2_t[:],
                                start=True, stop=True,
                            )
                        wlo = pend[0]
                        ytb = spool.tile([128, nwb * out_], F32, tag="ytb")
                        nc.vector.tensor_copy(
                            ytb[:].rearrange("p (c f) -> p c f", f=out_),
                            ypb[:],
                        )
                        nc.sync.dma_start(
                            out=y[wlo * 128 : (w + 1) * 128, :].rearrange(
                                "(c p) f -> p c f", p=128
                            ),
                            in_=ytb[:].rearrange("p (c f) -> p c f", f=out_),
                        )
                        pend = []
    nc.finalize()
    return nc


# ---------------------------------------------------------------- host glue
def _plan_windows(deg, npc, nw, ncores):
    """Per-core node->window assignment + uniform per-window chunk counts."""
    orders = []
    nch = np.zeros(nw, np.int64)
    for c in range(ncores):
        dl = deg[c * npc : (c + 1) * npc]
        order = np.argsort(-dl, kind="stable")
        orders.append(order)
        dls = dl[order]
        for w in range(nw):
            s = slice(w * 128, (w + 1) * 128)
            if dls[s].size:
                nch[w] = max(nch[w], int(dls[s].max()))
    nch[nch == 0] = 1
    return orders, nch


def _make_groups(nch, nw, gmax):
    groups = []
    w0 = 0
    while w0 < nw:
        w1 = w0 + 1
        tot = int(nch[w0])
        while w1 < nw and tot + int(nch[w1]) <= gmax:
            tot += int(nch[w1])
            w1 += 1
        groups.append((w0, w1))
        w0 = w1
    return groups


def kernel(x, edge_index, W1, att_src, att_dst, W2):
    x = np.asarray(x, dtype=np.float32)
    edge_index = np.asarray(edge_index)
    W1 = np.asarray(W1, dtype=np.float32)
    att_src = np.asarray(att_src, dtype=np.float32)
    att_dst = np.asarray(att_dst, dtype=np.float32)
    W2 = np.asarray(W2, dtype=np.float32)

    src = edge_index[0].astype(np.int64)
    dst = edge_index[1].astype(np.int64)

    trace = os.environ.get("BASS_GAT_TRACE") == "1"
    tkw = dict(trace=True, trace_cores=[0]) if trace else {}
    if trace:
        _patch_perfetto()

    # ---- phase 1: sharded hT/es/ed compute (fp16)
    xT16 = np.ascontiguousarray(x.T.astype(np.float16))     # [IN, N]
    w1_16 = W1.astype(np.float16)
    att16 = np.stack([att_src, att_dst], axis=1).astype(np.float16)  # [HID,2]

    nc1 = build_phase1()
    in_maps1 = []
    for c in range(NCORES):
        sh = xT16[:, c * NPC : (c + 1) * NPC]
        if sh.shape[1] < NPAD:
            sh = np.concatenate(
                [sh, np.zeros((IN, NPAD - sh.shape[1]), np.float16)], axis=1
            )
        in_maps1.append(
            {"xT": np.ascontiguousarray(sh), "w1": w1_16, "att": att16}
        )
    t0 = time.time()
    res1 = run_bass_kernel_spmd(nc1, in_maps1, core_ids=list(range(NCORES)), **tkw)
    _timings["phase1_wall"] = time.time() - t0
    _timings["phase1_ns"] = res1.exec_time_ns

    h_ext = np.zeros((N + 1, HID), np.float16)  # + zero dummy row for pads
    es_all = np.empty(N, np.float32)
    ed_all = np.empty(N, np.float32)
    for c in range(NCORES):
        sl = slice(c * NPC, (c + 1) * NPC)
        h_ext[sl] = res1.results[c]["hTo"][:, :NPC].T
        es_all[sl] = res1.results[c]["eso"][0, :NPC]
        ed_all[sl] = res1.results[c]["eso"][1, :NPC]

    # ---- host edge routing + halo pre-gather
    deg = np.bincount(dst, minlength=N)
    orders, nch = _plan_windows(deg, NPC, NW, NCORES)
    groups = _make_groups(nch, NW, GMAX)
    TOT = int(nch.sum())
    offs = np.zeros(NW + 1, np.int64)
    offs[1:] = np.cumsum(nch)

    eorder = np.argsort(dst, kind="stable")
    src_s = src[eorder]
    es_edge = es_all[src_s]
    estarts = np.zeros(N + 1, np.int64)
    estarts[1:] = np.cumsum(deg)

    w2_16 = W2.astype(np.float16)
    in_maps2 = []
    for c in range(NCORES):
        order = orders[c]
        idx32 = np.full((128, TOT), N, np.int64)   # N -> zero dummy row
        lgv = np.full((128, TOT), -30.0, np.float32)
        pcwv = np.zeros((128, NW), np.float32)
        for w in range(NW):
            nodes = order[w * 128 : (w + 1) * 128]
            o = int(offs[w])
            for p, j in enumerate(nodes):
                g = c * NPC + j
                s0, d = int(estarts[g]), int(deg[g])
                idx32[p, o : o + d] = src_s[s0 : s0 + d]
                lgv[p, o : o + d] = es_edge[s0 : s0 + d] + ed_all[g]
                pcwv[p, w] = nch[w] - d
            for p in range(len(nodes), 128):
                pcwv[p, w] = nch[w]
        gat = h_ext[idx32]                          # [128, TOT, HID] fp16
        in_maps2.append(
            {
                "gat": np.ascontiguousarray(gat.reshape(128, TOT * HID)),
                "lg": lgv,
                "pcw": pcwv,
                "w2": w2_16,
            }
        )

    nc2 = build_phase2(nch, groups)
    t0 = time.time()
    res2 = run_bass_kernel_spmd(nc2, in_maps2, core_ids=list(range(NCORES)), **tkw)
    _timings["phase2_wall"] = time.time() - t0
    _timings["phase2_ns"] = res2.exec_time_ns

    out = np.zeros((N, OUT), np.float32)
    for c in range(NCORES):
        yv = res2.results[c]["y"]
        order = orders[c]
        out[c * NPC + order] = yv[:NPC]
    return out
